# revision 11
# baseline (speedup 1.0000x reference)
"""Trainium2 Bass kernel for nn_ContrastByClassCalculator (MoCo-style
per-class-queue contrastive loss).

Math (reference):
    l_pos[n]  = q[n] . k[n]                                  # [N, 1]
    l_neg[n,:] = q[n] @ queue[cls_labels[n]]                 # [N, K]
    logits = concat([l_pos, l_neg], 1) / T                   # [N, 1+K]
    loss = mean_n( -log_softmax(logits)[n, 0] )

Strategy (v3):
  * Shard the queue over classes: 13 class slabs per core (8 cores,
    100 classes, 4 cores carry a zero-weight duplicate slab).  The
    class->core assignment is load-balanced on sample count at runtime.
  * Everything ships in fp8 e4m3 (queue slabs + packed q vectors):
    halves HBM traffic vs bf16 at 1.6e-4 relative loss error.  Slabs
    are pre-transposed on host to [D, SLOTS*K] so every DMA descriptor
    is one contiguous run per partition.
  * Dense PSUM packing via interleaved accumulation: each class's
    matmul uses a stationary that is zero except its own samples'
    columns, all accumulating into a shared PSUM tile.  ~62 real
    sample rows per core instead of 32-row per-class slots.
  * Four pipeline units sized to exactly fill PSUM (16 KB/partition):
      U0: 6 classes, unfolded  [128, 2048]  (8 KB)
      U1: 4 classes, fold 2    [128, 1024]  (4 KB)  k-half j -> rows 64j+
      U2: 2 classes, fold 4    [128,  512]  (2 KB)  k-quarter j -> rows 32j+
      U3: 1 class,   fold 4    [128,  512]  (2 KB)
    Each unit's row-max (DVE) and exp-sum (ACT) run while the PE is
    still streaming later units, and the folded tail units make the
    post-last-matmul chain 512-wide instead of 2048-wide.
  * The PE is pre-warmed with dummy matmuls during the DMA fill so the
    HAM clock gate (1.2 -> 2.4 GHz after ~3.4 us of activity) flips
    before the real matmuls start.
  * Device outputs only per-row (row_max, sum_exp) partials [128, 8];
    the host computes the positive logits, the online-softmax merge
    across fold blocks, the log, and the final mean in float64.
"""

import os

import numpy as np

import concourse.bacc as bacc
import concourse.mybir as mybir
import concourse.tile as tile
from concourse import bass_utils

# Problem constants (hardcoded per contract; kernel.py must be self-contained)
N = 512
D = 128
C = 100
K = 2048
T = 0.07
INV_T = float(1.0 / T)

N_CORES = 8
SLOTS = 13

# pipeline units: (n_classes, fold, stationary width = row capacity)
UNITS = [
    (6, 1, 128),
    (4, 2, 64),
    (2, 4, 32),
    (1, 4, 32),
]
assert sum(u[0] for u in UNITS) == SLOTS
N_UNITS = len(UNITS)

# slab DMA chunks (slot ranges): first chunk 1 slab for the earliest
# possible PE start; last chunks small so the tail is gated by little data.
CHUNKS = [(0, 1), (1, 3), (3, 6), (6, 9), (9, 11), (11, 12), (12, 13)]

# qt column layout: per-unit, per-class stationary blocks
_qo = 0
UNIT_QOFF = []
for _nc_, _f, _w in UNITS:
    UNIT_QOFF.append(_qo)
    _qo += _nc_ * _w
QW = _qo

N_WARM_MM = 8  # dummy matmuls to flip the PE HAM clock gate during DMA fill

FP32 = mybir.dt.float32
BF16 = mybir.dt.bfloat16
FP8 = mybir.dt.float8e4  # TRN FP8_EXP4 == ml_dtypes.float8_e4m3

# Results of the last hardware run (for test harnesses): BassKernelResults
last_run = None


def _build_nc():
    """Single-core SPMD Bass/Tile program."""
    nc = bacc.Bacc("TRN2")

    qt_h = nc.dram_tensor("qt", [D, QW], FP8, kind="ExternalInput")
    slabs_h = nc.dram_tensor("slabs", [D, SLOTS * K], FP8, kind="ExternalInput")
    out_h = nc.dram_tensor("out", [128, 2 * N_UNITS], FP32, kind="ExternalOutput")

    AX = mybir.AxisListType
    AF = mybir.ActivationFunctionType

    with tile.TileContext(nc) as tc:
        with (
            tc.tile_pool(name="consts", bufs=1) as consts,
            tc.tile_pool(name="small", bufs=1) as small,
            tc.tile_pool(name="slab", bufs=1) as slab_pool,
            tc.tile_pool(name="esc", bufs=1) as esc_pool,
            tc.tile_pool(name="psum", bufs=1, space="PSUM") as psum_pool,
        ):
            # qt first (needed by the first LDWEIGHTS), then slab chunks in
            # processing order.  One HWDGE dispatch each; descriptors are one
            # contiguous run per partition.
            qt = consts.tile([D, QW], FP8)
            nc.sync.dma_start(out=qt[:], in_=qt_h[:])
            slab_tiles = {}  # slot -> (tile, col offset)
            for c0, c1 in CHUNKS:
                st = slab_pool.tile([D, (c1 - c0) * K], FP8, tag=f"slab{c0}")
                nc.sync.dma_start(out=st[:], in_=slabs_h[:, c0 * K:c1 * K])
                for t in range(c0, c1):
                    slab_tiles[t] = (st, (t - c0) * K)

            # per-unit PSUM tiles (16 KB/partition total)
            ps = []
            for u, (_n, _f, _w) in enumerate(UNITS):
                pst = psum_pool.tile([128, K // _f], FP32, tag=f"ps{u}", name=f"ps{u}")
                ps.append(pst)

            # PE pre-warm: dummy matmuls on a zeroed tile into the last
            # (smallest) PSUM tile, which the real unit overwrites later.
            dummy = small.tile([128, 512], FP8)
            nc.vector.memset(dummy[:], 0.0)
            for _ in range(N_WARM_MM):
                nc.tensor.matmul(
                    out=ps[-1][0:32, :],
                    lhsT=dummy[:, 0:32],
                    rhs=dummy[:, :],
                    start=True,
                    stop=True,
                    tile_position=(0, 0),
                    skip_group_check=True,
                )

            # Warm the Exp spline table while the DMAs stream.
            warm = small.tile([1, 1], FP32)
            nc.vector.memset(warm[:], 0.0)
            nc.scalar.activation(out=warm[:], in_=warm[:], func=AF.Exp)

            # osb columns: (row_max, sum_exp) per unit
            osb = small.tile([128, 2 * N_UNITS], FP32)

            slot = 0
            for u, (n_cls, fold, width) in enumerate(UNITS):
                kf = K // fold          # unit free dim
                rb = 128 // fold        # fold-block row stride
                for ci in range(n_cls):
                    st, off = slab_tiles[slot]
                    lhsT = qt[:, UNIT_QOFF[u] + width * ci:
                              UNIT_QOFF[u] + width * (ci + 1)]
                    for j in range(fold):
                        for m in range(kf // 512):
                            nc.tensor.matmul(
                                out=ps[u][rb * j:rb * j + width,
                                          512 * m:512 * (m + 1)],
                                lhsT=lhsT,
                                rhs=st[:, off + kf * j + 512 * m:
                                       off + kf * j + 512 * (m + 1)],
                                start=(ci == 0),
                                stop=(ci == n_cls - 1),
                                tile_position=(0, rb * j),
                                # fold blocks are partition-disjoint groups in
                                # shared banks; the sim's tracker is
                                # partition-blind
                                skip_group_check=True,
                            )
                    slot += 1

                nc.vector.reduce_max(
                    out=osb[:, 2 * u:2 * u + 1], in_=ps[u][:], axis=AX.X
                )
                bias = small.tile([128, 1], FP32, tag=f"bias{u}")
                nc.gpsimd.tensor_scalar_mul(
                    out=bias[:], in0=osb[:, 2 * u:2 * u + 1], scalar1=-INV_T
                )
                esc = esc_pool.tile([128, kf], BF16, tag=f"esc{u}")
                nc.scalar.activation(
                    out=esc[:],
                    in_=ps[u][:],
                    func=AF.Exp,
                    bias=bias[:],
                    scale=INV_T,
                    accum_out=osb[:, 2 * u + 1:2 * u + 2],
                )

            nc.sync.dma_start(out=out_h[:], in_=osb[:])

    return nc


def _assign_classes(cls_labels):
    """Load-balanced class->core assignment.

    Returns per-core plans: ordered slot class list (with duplicates for
    12-class cores) and per-unit class lists.
    """
    counts = np.bincount(cls_labels, minlength=C)
    caps = [13, 13, 13, 13, 12, 12, 12, 12]
    order = np.argsort(-counts, kind="stable")
    cores = [[] for _ in range(N_CORES)]
    rows = [0] * N_CORES
    for c in order:
        cand = [i for i in range(N_CORES) if len(cores[i]) < caps[i]]
        i = min(cand, key=lambda i: (rows[i], i))
        cores[i].append(int(c))
        rows[i] += int(counts[c])

    plans = []
    for i in range(N_CORES):
        cl = sorted(cores[i], key=lambda c: (-counts[c], c))
        n_dup = SLOTS - len(cl)
        # units are carved off the count-sorted real list; duplicate slabs
        # (zero stationary, no samples) pad the END of unit 0
        units, pos = [], 0
        for u, (n_cls, fold, width) in enumerate(UNITS):
            n_real = n_cls - n_dup if u == 0 else n_cls
            units.append(cl[pos:pos + n_real] + [cl[0]] * (n_cls - n_real))
            pos += n_real
        assert pos == len(cl)
        slots = [c for ucl in units for c in ucl]
        plans.append({"slots": slots, "units": units, "n_dup": n_dup})
    return plans


def _pack_inputs(q, queue, cls_labels, plans):
    """Per-core fp8 packing: transposed slab windows + masked stationaries.

    Returns (in_maps, row_maps); row_maps[i][u] = list of sample indices in
    unit u's fold-block row order.
    """
    import ml_dtypes

    qf8 = np.ascontiguousarray(q).astype(ml_dtypes.float8_e4m3)
    in_maps, row_maps = [], []
    for i in range(N_CORES):
        p = plans[i]
        qt = np.zeros((D, QW), dtype=ml_dtypes.float8_e4m3)
        rows_per_unit = []
        n_dup = p["n_dup"]
        for u, (n_cls, fold, width) in enumerate(UNITS):
            urows = []
            n_real = n_cls - n_dup if u == 0 else n_cls
            for ci, c in enumerate(p["units"][u][:n_real]):
                for n in np.nonzero(cls_labels == c)[0]:
                    qt[:, UNIT_QOFF[u] + width * ci + len(urows)] = qf8[n]
                    urows.append(int(n))
            if len(urows) > width:
                raise ValueError(f"core {i} unit {u}: {len(urows)} rows > {width}")
            rows_per_unit.append(urows)

        # [SLOTS, D, K] -> [D, SLOTS*K] contiguous per partition
        slabs = np.ascontiguousarray(
            queue[p["slots"]].transpose(1, 0, 2).reshape(D, SLOTS * K)
        ).astype(ml_dtypes.float8_e4m3)

        in_maps.append({"qt": qt, "slabs": slabs})
        row_maps.append(rows_per_unit)
    return in_maps, row_maps


def _combine(outs, row_maps, lpos):
    """Host-side float64 merge of per-core device partials -> loss sum."""
    total = 0.0
    for i in range(N_CORES):
        o = np.asarray(outs[i], np.float64)
        for u, (n_cls, fold, width) in enumerate(UNITS):
            rb = 128 // fold
            for r, n in enumerate(row_maps[i][u]):
                ms = o[rb * np.arange(fold) + r, 2 * u]
                ss = o[rb * np.arange(fold) + r, 2 * u + 1]
                mt = max(ms.max(), lpos[n])
                denom = (ss * np.exp((ms - mt) * INV_T)).sum() + np.exp(
                    (lpos[n] - mt) * INV_T
                )
                total += np.log(denom) + (mt - lpos[n]) * INV_T
    return total


def kernel(q, k, queue, class_weights, cls_labels):
    global last_run
    q = np.asarray(q, dtype=np.float32)
    k = np.asarray(k, dtype=np.float32)
    queue = np.asarray(queue, dtype=np.float32)[:C]
    cls_labels = np.asarray(cls_labels).astype(np.int64)

    plans = _assign_classes(cls_labels)
    in_maps, row_maps = _pack_inputs(q, queue, cls_labels, plans)
    nc = _build_nc()
    if not nc.is_finalized():
        nc.finalize()

    trace = bool(os.environ.get("BASS_TRACE"))
    res = bass_utils.run_bass_kernel_spmd(
        nc, in_maps, list(range(N_CORES)), trace=trace
    )
    last_run = res

    lpos = (q.astype(np.float64) * k.astype(np.float64)).sum(1)
    total = _combine([r["out"] for r in res.results], row_maps, lpos)
    return np.float32(total / N)


# revision 19
# speedup vs baseline: 1.1504x; 1.1504x over previous
"""Trainium2 Bass kernel for nn_ContrastByClassCalculator (MoCo-style
per-class-queue contrastive loss).

Math (reference):
    l_pos[n]  = q[n] . k[n]                                  # [N, 1]
    l_neg[n,:] = q[n] @ queue[cls_labels[n]]                 # [N, K]
    logits = concat([l_pos, l_neg], 1) / T                   # [N, 1+K]
    loss = mean_n( -log_softmax(logits)[n, 0] )

Strategy (v4):
  * Shard the queue over classes: 13 class slabs per core (8 cores,
    100 classes, 4 cores carry a zero-weight duplicate slab).  The
    class->core assignment is load-balanced on sample count at runtime.
  * Everything ships in fp8 e4m3: halves HBM traffic vs bf16 at
    ~1.6e-4 relative loss error.  (No non-power-of-2 pre-scaling: it
    shifts values across fp8 binades and multiplies the error ~6x.)
    Slabs are pre-transposed on host to [D, SLOTS*K] so every DMA
    descriptor is one contiguous run per partition.
  * Dense PSUM packing via interleaved accumulation: each class's
    matmul uses a stationary that is zero except its own samples'
    columns, all accumulating into a shared PSUM tile.  Fold blocks
    (k-range j -> partition rows rb*j) exploit the unused partition
    rows to shrink the per-unit free dim, which is what the DVE
    row-max and ACT exp passes pay for:
      U0: 6 classes, fold 2  [128, 1024]  (4 KB PSUM)
      U1: 4 classes, fold 4  [128,  512]  (2 KB)
      U2: 2 classes, fold 4  [128,  512]  (2 KB)
      U3: 1 class,   fold 4  [128,  512]  (2 KB)
    Units pipeline: each unit's row-max (DVE, negate=True so the
    result IS the exp bias) and exp-sum (ACT accum) run while the PE
    streams later units.
  * The PE is pre-warmed with small dummy matmuls during the DMA fill
    so the HAM clock gate (1.2 -> 2.4 GHz) flips early.
  * Device outputs only per-row (-max/T, sum_exp) partials [128, 8];
    the host computes the positive logits, the online-softmax merge
    across fold blocks, the log, and the final mean in float64.
"""

import os

import numpy as np

import concourse.bacc as bacc
import concourse.mybir as mybir
import concourse.tile as tile
from concourse import bass_utils

# Problem constants (hardcoded per contract; kernel.py must be self-contained)
N = 512
D = 128
C = 100
K = 2048
T = 0.07
INV_T = float(1.0 / T)

N_CORES = 8
SLOTS = 13

# pipeline units: (n_classes, fold, stationary width = row capacity)
UNITS = [
    (6, 2, 64),
    (4, 4, 32),
    (2, 4, 32),
    (1, 4, 32),
]
assert sum(u[0] for u in UNITS) == SLOTS
N_UNITS = len(UNITS)

# slab DMA chunks (slot ranges): first chunk 1 slab for the earliest
# possible PE start; last chunks small so the tail is gated by little data.
CHUNKS = [(0, 1), (1, 3), (3, 6), (6, 9), (9, 11), (11, 12), (12, 13)]

# qt column layout: per-unit, per-class stationary blocks
_qo = 0
UNIT_QOFF = []
for _nc_, _f, _w in UNITS:
    UNIT_QOFF.append(_qo)
    _qo += _nc_ * _w
QW = _qo

N_WARM_MM = 8      # dummy matmuls to flip the PE HAM clock gate early
WARM_COLS = 256

FP32 = mybir.dt.float32
BF16 = mybir.dt.bfloat16
FP8 = mybir.dt.float8e4  # TRN FP8_EXP4 == ml_dtypes.float8_e4m3

# Results of the last hardware run (for test harnesses): BassKernelResults
last_run = None


def _build_nc():
    """Single-core SPMD Bass/Tile program."""
    nc = bacc.Bacc("TRN2")

    qt_h = nc.dram_tensor("qt", [D, QW], FP8, kind="ExternalInput")
    slabs_h = nc.dram_tensor("slabs", [D, SLOTS * K], FP8, kind="ExternalInput")
    out_h = nc.dram_tensor("out", [128, 2 * N_UNITS], FP32, kind="ExternalOutput")

    AX = mybir.AxisListType
    AF = mybir.ActivationFunctionType

    with tile.TileContext(nc) as tc:
        with (
            tc.tile_pool(name="consts", bufs=1) as consts,
            tc.tile_pool(name="small", bufs=1) as small,
            tc.tile_pool(name="slab", bufs=1) as slab_pool,
            tc.tile_pool(name="esc", bufs=1) as esc_pool,
            tc.tile_pool(name="psum", bufs=1, space="PSUM") as psum_pool,
        ):
            # qt first (needed by the first LDWEIGHTS), then slab chunks in
            # processing order.  One HWDGE dispatch each; descriptors are one
            # contiguous run per partition.
            qt = consts.tile([D, QW], FP8)
            nc.sync.dma_start(out=qt[:], in_=qt_h[:])
            slab_tiles = {}  # slot -> (tile, col offset)
            for c0, c1 in CHUNKS:
                st = slab_pool.tile([D, (c1 - c0) * K], FP8, tag=f"slab{c0}")
                nc.sync.dma_start(out=st[:], in_=slabs_h[:, c0 * K:c1 * K])
                for t in range(c0, c1):
                    slab_tiles[t] = (st, (t - c0) * K)

            # per-unit PSUM tiles (10 KB/partition total)
            ps = []
            for u, (_n, _f, _w) in enumerate(UNITS):
                pst = psum_pool.tile([128, K // _f], FP32, tag=f"ps{u}", name=f"ps{u}")
                ps.append(pst)

            # PE pre-warm: small dummy matmuls on a zeroed tile into the last
            # PSUM tile, which the real unit overwrites later.
            dummy = small.tile([128, WARM_COLS], FP8)
            nc.vector.memset(dummy[:], 0.0)
            for _ in range(N_WARM_MM):
                nc.tensor.matmul(
                    out=ps[-1][0:32, 0:WARM_COLS],
                    lhsT=dummy[:, 0:32],
                    rhs=dummy[:, :],
                    start=True,
                    stop=True,
                    tile_position=(0, 0),
                    skip_group_check=True,
                )

            # Warm the Exp spline table while the DMAs stream.
            warm = small.tile([1, 1], FP32)
            nc.vector.memset(warm[:], 0.0)
            nc.scalar.activation(out=warm[:], in_=warm[:], func=AF.Exp)

            # osb columns per unit: 2u = row max, 2u+1 = sum_exp
            osb = small.tile([128, 2 * N_UNITS], FP32)

            slot = 0
            for u, (n_cls, fold, width) in enumerate(UNITS):
                kf = K // fold          # unit free dim
                rb = 128 // fold        # fold-block row stride
                for ci in range(n_cls):
                    st, off = slab_tiles[slot]
                    lhsT = qt[:, UNIT_QOFF[u] + width * ci:
                              UNIT_QOFF[u] + width * (ci + 1)]
                    for j in range(fold):
                        for m in range(kf // 512):
                            nc.tensor.matmul(
                                out=ps[u][rb * j:rb * j + width,
                                          512 * m:512 * (m + 1)],
                                lhsT=lhsT,
                                rhs=st[:, off + kf * j + 512 * m:
                                       off + kf * j + 512 * (m + 1)],
                                start=(ci == 0),
                                stop=(ci == n_cls - 1),
                                tile_position=(0, rb * j),
                                # fold blocks are partition-disjoint groups in
                                # shared banks; the sim's tracker is
                                # partition-blind
                                skip_group_check=True,
                            )
                    slot += 1

                nc.vector.reduce_max(
                    out=osb[:, 2 * u:2 * u + 1], in_=ps[u][:], axis=AX.X
                )
                bias = small.tile([128, 1], FP32, tag=f"bias{u}", name=f"bias{u}")
                nc.vector.tensor_scalar_mul(
                    out=bias[:], in0=osb[:, 2 * u:2 * u + 1], scalar1=-INV_T
                )
                esc = esc_pool.tile([128, kf], BF16, tag=f"esc{u}")
                nc.scalar.activation(
                    out=esc[:],
                    in_=ps[u][:],
                    func=AF.Exp,
                    bias=bias[:],
                    scale=INV_T,
                    accum_out=osb[:, 2 * u + 1:2 * u + 2],
                )

            nc.sync.dma_start(out=out_h[:], in_=osb[:])

    return nc


def _assign_classes(cls_labels):
    """Load-balanced class->core assignment.

    Returns per-core plans: ordered slot class list (with duplicates for
    12-class cores) and per-unit class lists.
    """
    counts = np.bincount(cls_labels, minlength=C)
    caps = [13, 13, 13, 13, 12, 12, 12, 12]
    order = np.argsort(-counts, kind="stable")
    cores = [[] for _ in range(N_CORES)]
    rows = [0] * N_CORES
    for c in order:
        cand = [i for i in range(N_CORES) if len(cores[i]) < caps[i]]
        i = min(cand, key=lambda i: (rows[i], i))
        cores[i].append(int(c))
        rows[i] += int(counts[c])

    plans = []
    for i in range(N_CORES):
        cl = sorted(cores[i], key=lambda c: (-counts[c], c))
        n_dup = SLOTS - len(cl)
        # units are carved off the count-sorted real list; duplicate slabs
        # (zero stationary, no samples) pad the END of unit 0
        units, pos = [], 0
        for u, (n_cls, fold, width) in enumerate(UNITS):
            n_real = n_cls - n_dup if u == 0 else n_cls
            units.append(cl[pos:pos + n_real] + [cl[0]] * (n_cls - n_real))
            pos += n_real
        assert pos == len(cl)

        # repair row-capacity overflows by swapping classes between units
        # (count-desc carving can exceed a cap when counts are skewed)
        def unit_rows(u):
            n_real = UNITS[u][0] - n_dup if u == 0 else UNITS[u][0]
            return sum(int(counts[c]) for c in units[u][:n_real])

        for u in range(N_UNITS - 1):
            guard = 0
            while unit_rows(u) > UNITS[u][2] and guard < 64:
                guard += 1
                n_real = UNITS[u][0] - n_dup if u == 0 else UNITS[u][0]
                big = max(range(n_real), key=lambda j: counts[units[u][j]])
                small_j = min(
                    range(UNITS[u + 1][0]), key=lambda j: counts[units[u + 1][j]]
                )
                units[u][big], units[u + 1][small_j] = (
                    units[u + 1][small_j],
                    units[u][big],
                )
        for u in range(N_UNITS):
            if unit_rows(u) > UNITS[u][2]:
                raise ValueError(f"core {i} unit {u}: rows exceed capacity")

        slots = [c for ucl in units for c in ucl]
        plans.append({"slots": slots, "units": units, "n_dup": n_dup})
    return plans


def _pack_inputs(q, queue, cls_labels, plans):
    """Per-core fp8 packing: transposed slab windows + masked stationaries.

    Returns (in_maps, row_maps); row_maps[i][u] = list of sample indices in
    unit u's fold-block row order.
    """
    import ml_dtypes

    qf8 = np.ascontiguousarray(q).astype(ml_dtypes.float8_e4m3)
    in_maps, row_maps = [], []
    for i in range(N_CORES):
        p = plans[i]
        qt = np.zeros((D, QW), dtype=ml_dtypes.float8_e4m3)
        rows_per_unit = []
        n_dup = p["n_dup"]
        for u, (n_cls, fold, width) in enumerate(UNITS):
            urows = []
            n_real = n_cls - n_dup if u == 0 else n_cls
            for ci, c in enumerate(p["units"][u][:n_real]):
                for n in np.nonzero(cls_labels == c)[0]:
                    qt[:, UNIT_QOFF[u] + width * ci + len(urows)] = qf8[n]
                    urows.append(int(n))
            if len(urows) > width:
                raise ValueError(f"core {i} unit {u}: {len(urows)} rows > {width}")
            rows_per_unit.append(urows)

        # [SLOTS, D, K] -> [D, SLOTS*K] contiguous per partition
        slabs = np.ascontiguousarray(
            queue[p["slots"]].transpose(1, 0, 2).reshape(D, SLOTS * K)
        ).astype(ml_dtypes.float8_e4m3)

        in_maps.append({"qt": qt, "slabs": slabs})
        row_maps.append(rows_per_unit)
    return in_maps, row_maps


def _combine(outs, row_maps, lpos):
    """Host-side float64 merge of per-core device partials -> loss sum.

    Device columns per unit: col 2u = row max (logit units), 2u+1 = sum_exp
    of exp((l - max)/T).
    """
    total = 0.0
    for i in range(N_CORES):
        o = np.asarray(outs[i], np.float64)
        for u, (n_cls, fold, width) in enumerate(UNITS):
            rb = 128 // fold
            for r, n in enumerate(row_maps[i][u]):
                ms = o[rb * np.arange(fold) + r, 2 * u]       # max(l) per block
                ss = o[rb * np.arange(fold) + r, 2 * u + 1]
                mt = max(ms.max(), lpos[n])
                denom = (ss * np.exp((ms - mt) * INV_T)).sum() + np.exp(
                    (lpos[n] - mt) * INV_T
                )
                total += np.log(denom) + (mt - lpos[n]) * INV_T
    return total


def kernel(q, k, queue, class_weights, cls_labels):
    global last_run
    q = np.asarray(q, dtype=np.float32)
    k = np.asarray(k, dtype=np.float32)
    queue = np.asarray(queue, dtype=np.float32)[:C]
    cls_labels = np.asarray(cls_labels).astype(np.int64)

    plans = _assign_classes(cls_labels)
    in_maps, row_maps = _pack_inputs(q, queue, cls_labels, plans)
    nc = _build_nc()
    if not nc.is_finalized():
        nc.finalize()

    trace = bool(os.environ.get("BASS_TRACE"))
    res = bass_utils.run_bass_kernel_spmd(
        nc, in_maps, list(range(N_CORES)), trace=trace
    )
    last_run = res

    lpos = (q.astype(np.float64) * k.astype(np.float64)).sum(1)
    total = _combine([r["out"] for r in res.results], row_maps, lpos)
    return np.float32(total / N)


# revision 22
# speedup vs baseline: 1.2094x; 1.0512x over previous
"""Trainium2 Bass kernel for nn_ContrastByClassCalculator (MoCo-style
per-class-queue contrastive loss).

Math (reference):
    l_pos[n]  = q[n] . k[n]                                  # [N, 1]
    l_neg[n,:] = q[n] @ queue[cls_labels[n]]                 # [N, K]
    logits = concat([l_pos, l_neg], 1) / T                   # [N, 1+K]
    loss = mean_n( -log_softmax(logits)[n, 0] )

Strategy (v4):
  * Shard the queue over classes: 13 class slabs per core (8 cores,
    100 classes, 4 cores carry a zero-weight duplicate slab).  The
    class->core assignment is load-balanced on sample count at runtime.
  * Everything ships in fp8 e4m3: halves HBM traffic vs bf16 at
    ~1.6e-4 relative loss error.  (No non-power-of-2 pre-scaling: it
    shifts values across fp8 binades and multiplies the error ~6x.)
    Slabs are pre-transposed on host to [D, SLOTS*K] so every DMA
    descriptor is one contiguous run per partition.
  * Dense PSUM packing via interleaved accumulation: each class's
    matmul uses a stationary that is zero except its own samples'
    columns, all accumulating into a shared PSUM tile.  Fold blocks
    (k-range j -> partition rows rb*j) exploit the unused partition
    rows to shrink the per-unit free dim, which is what the DVE
    row-max and ACT exp passes pay for:
      U0: 4 classes, fold 2  [128, 1024]  (4 KB PSUM)
      U1-U3: 3/3/2 classes, fold 4  [128, 512]  (2 KB each)
      U4: 1 class,  fold 4  [128,  512]  (2 KB)
    Units pipeline: each unit's row-max (DVE) and exp-sum (ACT accum)
    run while the PE streams later units; DMA chunk boundaries align
    with unit boundaries.
  * The PE is pre-warmed with small dummy matmuls during the DMA fill
    so the HAM clock gate (1.2 -> 2.4 GHz) flips early.
  * Device outputs only per-row (row_max, sum_exp) partials [128, 10];
    the host computes the positive logits, the online-softmax merge
    across fold blocks, the log, and the final mean in float64.
"""

import os

import numpy as np

import concourse.bacc as bacc
import concourse.mybir as mybir
import concourse.tile as tile
from concourse import bass_utils

# Problem constants (hardcoded per contract; kernel.py must be self-contained)
N = 512
D = 128
C = 100
K = 2048
T = 0.07
INV_T = float(1.0 / T)

N_CORES = 8
SLOTS = 13

# pipeline units: (n_classes, fold, stationary width = row capacity)
UNITS = [
    (4, 2, 64),
    (3, 4, 32),
    (3, 4, 32),
    (2, 4, 32),
    (1, 4, 32),
]
assert sum(u[0] for u in UNITS) == SLOTS
N_UNITS = len(UNITS)

# slab DMA chunks (slot ranges): first chunk 1 slab for the earliest
# possible PE start; later boundaries align with unit boundaries so each
# unit's row-max/exp chain starts as soon as its own slabs land.
CHUNKS = [(0, 1), (1, 4), (4, 7), (7, 10), (10, 12), (12, 13)]

# qt column layout: per-unit, per-class stationary blocks
_qo = 0
UNIT_QOFF = []
for _nc_, _f, _w in UNITS:
    UNIT_QOFF.append(_qo)
    _qo += _nc_ * _w
QW = _qo

N_WARM_MM = 12     # dummy matmuls to flip the PE HAM clock gate early
WARM_COLS = 256

FP32 = mybir.dt.float32
BF16 = mybir.dt.bfloat16
FP8 = mybir.dt.float8e4  # TRN FP8_EXP4 == ml_dtypes.float8_e4m3

# Results of the last hardware run (for test harnesses): BassKernelResults
last_run = None


def _build_nc():
    """Single-core SPMD Bass/Tile program."""
    nc = bacc.Bacc("TRN2")

    qt_h = nc.dram_tensor("qt", [D, QW], FP8, kind="ExternalInput")
    slabs_h = nc.dram_tensor("slabs", [D, SLOTS * K], FP8, kind="ExternalInput")
    out_h = nc.dram_tensor("out", [128, 2 * N_UNITS], FP32, kind="ExternalOutput")

    AX = mybir.AxisListType
    AF = mybir.ActivationFunctionType

    with tile.TileContext(nc) as tc:
        with (
            tc.tile_pool(name="consts", bufs=1) as consts,
            tc.tile_pool(name="small", bufs=1) as small,
            tc.tile_pool(name="slab", bufs=1) as slab_pool,
            tc.tile_pool(name="esc", bufs=1) as esc_pool,
            tc.tile_pool(name="psum", bufs=1, space="PSUM") as psum_pool,
        ):
            # qt first (needed by the first LDWEIGHTS), then slab chunks in
            # processing order.  One HWDGE dispatch each; descriptors are one
            # contiguous run per partition.
            qt = consts.tile([D, QW], FP8)
            nc.sync.dma_start(out=qt[:], in_=qt_h[:])
            slab_tiles = {}  # slot -> (tile, col offset)
            for c0, c1 in CHUNKS:
                st = slab_pool.tile([D, (c1 - c0) * K], FP8, tag=f"slab{c0}")
                nc.sync.dma_start(out=st[:], in_=slabs_h[:, c0 * K:c1 * K])
                for t in range(c0, c1):
                    slab_tiles[t] = (st, (t - c0) * K)

            # per-unit PSUM tiles (10 KB/partition total)
            ps = []
            for u, (_n, _f, _w) in enumerate(UNITS):
                pst = psum_pool.tile([128, K // _f], FP32, tag=f"ps{u}", name=f"ps{u}")
                ps.append(pst)

            # PE pre-warm: small dummy matmuls on a zeroed tile into the last
            # PSUM tile, which the real unit overwrites later.
            dummy = small.tile([128, WARM_COLS], FP8)
            nc.vector.memset(dummy[:], 0.0)
            for _ in range(N_WARM_MM):
                nc.tensor.matmul(
                    out=ps[-1][0:32, 0:WARM_COLS],
                    lhsT=dummy[:, 0:32],
                    rhs=dummy[:, :],
                    start=True,
                    stop=True,
                    tile_position=(0, 0),
                    skip_group_check=True,
                )

            # Warm the Exp spline table while the DMAs stream.
            warm = small.tile([1, 1], FP32)
            nc.vector.memset(warm[:], 0.0)
            nc.scalar.activation(out=warm[:], in_=warm[:], func=AF.Exp)

            # osb columns per unit: 2u = row max, 2u+1 = sum_exp
            osb = small.tile([128, 2 * N_UNITS], FP32)

            slot = 0
            for u, (n_cls, fold, width) in enumerate(UNITS):
                kf = K // fold          # unit free dim
                rb = 128 // fold        # fold-block row stride
                for ci in range(n_cls):
                    st, off = slab_tiles[slot]
                    lhsT = qt[:, UNIT_QOFF[u] + width * ci:
                              UNIT_QOFF[u] + width * (ci + 1)]
                    for j in range(fold):
                        for m in range(kf // 512):
                            nc.tensor.matmul(
                                out=ps[u][rb * j:rb * j + width,
                                          512 * m:512 * (m + 1)],
                                lhsT=lhsT,
                                rhs=st[:, off + kf * j + 512 * m:
                                       off + kf * j + 512 * (m + 1)],
                                start=(ci == 0),
                                stop=(ci == n_cls - 1),
                                tile_position=(0, rb * j),
                                # fold blocks are partition-disjoint groups in
                                # shared banks; the sim's tracker is
                                # partition-blind
                                skip_group_check=True,
                            )
                    slot += 1

                nc.vector.reduce_max(
                    out=osb[:, 2 * u:2 * u + 1], in_=ps[u][:], axis=AX.X
                )
                bias = small.tile([128, 1], FP32, tag=f"bias{u}", name=f"bias{u}")
                nc.vector.tensor_scalar_mul(
                    out=bias[:], in0=osb[:, 2 * u:2 * u + 1], scalar1=-INV_T
                )
                esc = esc_pool.tile([128, kf], BF16, tag=f"esc{u}")
                nc.scalar.activation(
                    out=esc[:],
                    in_=ps[u][:],
                    func=AF.Exp,
                    bias=bias[:],
                    scale=INV_T,
                    accum_out=osb[:, 2 * u + 1:2 * u + 2],
                )

            nc.sync.dma_start(out=out_h[:], in_=osb[:])

    return nc


def _assign_classes(cls_labels):
    """Load-balanced class->core assignment.

    Returns per-core plans: ordered slot class list (with duplicates for
    12-class cores) and per-unit class lists.
    """
    counts = np.bincount(cls_labels, minlength=C)
    caps = [13, 13, 13, 13, 12, 12, 12, 12]
    order = np.argsort(-counts, kind="stable")
    cores = [[] for _ in range(N_CORES)]
    rows = [0] * N_CORES
    for c in order:
        cand = [i for i in range(N_CORES) if len(cores[i]) < caps[i]]
        i = min(cand, key=lambda i: (rows[i], i))
        cores[i].append(int(c))
        rows[i] += int(counts[c])

    plans = []
    for i in range(N_CORES):
        cl = sorted(cores[i], key=lambda c: (-counts[c], c))
        n_dup = SLOTS - len(cl)
        # units are carved off the count-sorted real list; duplicate slabs
        # (zero stationary, no samples) pad the END of unit 0
        units, pos = [], 0
        for u, (n_cls, fold, width) in enumerate(UNITS):
            n_real = n_cls - n_dup if u == 0 else n_cls
            units.append(cl[pos:pos + n_real] + [cl[0]] * (n_cls - n_real))
            pos += n_real
        assert pos == len(cl)

        # repair row-capacity overflows by swapping classes between units
        # (count-desc carving can exceed a cap when counts are skewed)
        def unit_rows(u):
            n_real = UNITS[u][0] - n_dup if u == 0 else UNITS[u][0]
            return sum(int(counts[c]) for c in units[u][:n_real])

        for u in range(N_UNITS - 1):
            guard = 0
            while unit_rows(u) > UNITS[u][2] and guard < 64:
                guard += 1
                n_real = UNITS[u][0] - n_dup if u == 0 else UNITS[u][0]
                big = max(range(n_real), key=lambda j: counts[units[u][j]])
                small_j = min(
                    range(UNITS[u + 1][0]), key=lambda j: counts[units[u + 1][j]]
                )
                units[u][big], units[u + 1][small_j] = (
                    units[u + 1][small_j],
                    units[u][big],
                )
        for u in range(N_UNITS):
            if unit_rows(u) > UNITS[u][2]:
                raise ValueError(f"core {i} unit {u}: rows exceed capacity")

        slots = [c for ucl in units for c in ucl]
        plans.append({"slots": slots, "units": units, "n_dup": n_dup})
    return plans


def _pack_inputs(q, queue, cls_labels, plans):
    """Per-core fp8 packing: transposed slab windows + masked stationaries.

    Returns (in_maps, row_maps); row_maps[i][u] = list of sample indices in
    unit u's fold-block row order.
    """
    import ml_dtypes

    qf8 = np.ascontiguousarray(q).astype(ml_dtypes.float8_e4m3)
    in_maps, row_maps = [], []
    for i in range(N_CORES):
        p = plans[i]
        qt = np.zeros((D, QW), dtype=ml_dtypes.float8_e4m3)
        rows_per_unit = []
        n_dup = p["n_dup"]
        for u, (n_cls, fold, width) in enumerate(UNITS):
            urows = []
            n_real = n_cls - n_dup if u == 0 else n_cls
            for ci, c in enumerate(p["units"][u][:n_real]):
                for n in np.nonzero(cls_labels == c)[0]:
                    qt[:, UNIT_QOFF[u] + width * ci + len(urows)] = qf8[n]
                    urows.append(int(n))
            if len(urows) > width:
                raise ValueError(f"core {i} unit {u}: {len(urows)} rows > {width}")
            rows_per_unit.append(urows)

        # [SLOTS, D, K] -> [D, SLOTS*K] contiguous per partition
        slabs = np.ascontiguousarray(
            queue[p["slots"]].transpose(1, 0, 2).reshape(D, SLOTS * K)
        ).astype(ml_dtypes.float8_e4m3)

        in_maps.append({"qt": qt, "slabs": slabs})
        row_maps.append(rows_per_unit)
    return in_maps, row_maps


def _combine(outs, row_maps, lpos):
    """Host-side float64 merge of per-core device partials -> loss sum.

    Device columns per unit: col 2u = row max (logit units), 2u+1 = sum_exp
    of exp((l - max)/T).
    """
    total = 0.0
    for i in range(N_CORES):
        o = np.asarray(outs[i], np.float64)
        for u, (n_cls, fold, width) in enumerate(UNITS):
            rb = 128 // fold
            for r, n in enumerate(row_maps[i][u]):
                ms = o[rb * np.arange(fold) + r, 2 * u]       # max(l) per block
                ss = o[rb * np.arange(fold) + r, 2 * u + 1]
                mt = max(ms.max(), lpos[n])
                denom = (ss * np.exp((ms - mt) * INV_T)).sum() + np.exp(
                    (lpos[n] - mt) * INV_T
                )
                total += np.log(denom) + (mt - lpos[n]) * INV_T
    return total


def kernel(q, k, queue, class_weights, cls_labels):
    global last_run
    q = np.asarray(q, dtype=np.float32)
    k = np.asarray(k, dtype=np.float32)
    queue = np.asarray(queue, dtype=np.float32)[:C]
    cls_labels = np.asarray(cls_labels).astype(np.int64)

    plans = _assign_classes(cls_labels)
    in_maps, row_maps = _pack_inputs(q, queue, cls_labels, plans)
    nc = _build_nc()
    if not nc.is_finalized():
        nc.finalize()

    trace = bool(os.environ.get("BASS_TRACE"))
    res = bass_utils.run_bass_kernel_spmd(
        nc, in_maps, list(range(N_CORES)), trace=trace
    )
    last_run = res

    lpos = (q.astype(np.float64) * k.astype(np.float64)).sum(1)
    total = _combine([r["out"] for r in res.results], row_maps, lpos)
    return np.float32(total / N)
